# revision 20
# baseline (speedup 1.0000x reference)
"""MoE (top-2 of 8 experts) Trainium2 kernel.

Strategy: token-balanced expert loop over 8 NeuronCores. The router
(softmax + top-2 + renormalize) runs on host in f32 numpy, exactly
mirroring the jax reference semantics (stable argsort == lax.top_k
tie-breaking). Every core loops over all 8 experts; expert e's routed
tokens are dealt stride-8 across cores, so each core processes exactly
v_e = ceil(count_e/8) tokens of expert e — per-core work is balanced to
<0.1% regardless of routing skew. Expert weights are DMA-streamed per
expert (bf16, double-buffered, hidden under the ~55us of matmul per
expert). Core math per expert (combine-weight scaling and the w*b2
rank-1 term are applied on host, exactly, during the scatter-add):

    y = relu(x @ W1[e] + b1[e]) @ W2[e]

Matmuls run in bf16 on the PE array with f32 PSUM accumulation; b1-add +
relu is fused into one ScalarE activation. Tokens are processed in
blocks of <=512 (one PSUM bank). Full 128-token groups go through the
token-partition stage 2 (y layout [tok, d]); the sub-128 remainder of
each block goes through a transposed stage 2 (tokens as the moving dim,
cost proportional to the remainder) writing y^T to a side tensor that
the host transposes back. Stage 1 of block k+1 is emitted before stage
2 of block k so the PE stream never stalls on the relu drain. w1 is
fed in 64 [128,128] chunks so the first matmul starts after ~160KB of
DMA instead of 2MB.

Layouts (host-prepped so the device only does natural slices):
  xT  [4,128,C]        bf16  x_gathered^T as (d//128, d%128, slot)
  w1  [E,16,4,128,128] bf16  W1 as (e, f//128, d//128, d%128, f%128)
  w2  [E,16,128,D]     bf16  W2 as (e, f//128, f%128, d)
  b1  [E,128,16]       f32   b1 as (e, f%128, f//128) -> ACT bias column
  y   [C,D]            f32   full 128-token groups, [slot, d]
  yt  [4,128,R]        f32   remainder groups, (d//128, d%128, rem-slot)
"""

import numpy as np
import ml_dtypes

import concourse.bass as bass
import concourse.mybir as mybir
import concourse.tile as tile
from concourse import bacc, bass_utils

B, S, D, F, E, TOPK = 64, 512, 512, 2048, 8, 2
N_CORES = 8
TOK_BLK = 512

_BF16 = ml_dtypes.bfloat16
_compiled_cache: dict[tuple, "bacc.Bacc"] = {}
LAST_RESULTS = None  # test harness reads exec_time_ns / profile from here


def _block_list(shares):
    """Compile-time blocks: (expert, slot_off, n_tok, rem_off).

    n_tok <= 512; rem_off is the running column in the transposed
    remainder tensor for this block's n_tok % 128 tail (or None).
    """
    blocks = []
    off = 0
    roff = 0
    for e, sh in enumerate(shares):
        left = sh
        while left > 0:
            n = min(TOK_BLK, left)
            r = n % 128
            blocks.append((e, off, n, roff if r else None))
            roff += r
            off += n
            left -= n
    return blocks, off, roff


def _build_kernel(shares) -> "bacc.Bacc":
    blocks, C, R = _block_list(shares)
    nc = bacc.Bacc("TRN2", target_bir_lowering=False, debug=False,
                   num_devices=N_CORES)

    xT_d = nc.dram_tensor("xT", [4, 128, C], mybir.dt.bfloat16,
                          kind="ExternalInput")
    w1_d = nc.dram_tensor("w1", [E, 4, 4, 128, 512], mybir.dt.bfloat16,
                          kind="ExternalInput")
    w2_d = nc.dram_tensor("w2", [E, 16, 128, D], mybir.dt.bfloat16,
                          kind="ExternalInput")
    b1_d = nc.dram_tensor("b1", [E, 128, 16], mybir.dt.float32,
                          kind="ExternalInput")
    y_d = nc.dram_tensor("y", [C, D], mybir.dt.float32,
                         kind="ExternalOutput")

    with tile.TileContext(nc) as tc:
        with (
            tc.tile_pool(name="wpool", bufs=2) as wpool,
            tc.tile_pool(name="xin", bufs=6) as xpool,
            tc.tile_pool(name="hbuf", bufs=2) as hpool,
            tc.tile_pool(name="yout", bufs=3) as ypool,
            tc.tile_pool(name="ph", bufs=3, space="PSUM") as phpool,
            tc.tile_pool(name="py", bufs=3, space="PSUM") as pypool,
        ):
            def load_expert(e, xt0=None):
                w1_sb, w2_sb = [], []
                for j2 in range(4):
                    row = []
                    for i in range(4):
                        t = wpool.tile([128, 512], mybir.dt.bfloat16,
                                       tag=f"w1_{j2}_{i}", name=f"w1_{j2}_{i}")
                        # first expert: first x block goes down the gpsimd
                        # queue in parallel with w1 on sync, so the PE's
                        # first matmul (needs only w1_00 + xt_0) starts early
                        if xt0 is not None and j2 == 0:
                            nc.gpsimd.dma_start(
                                xt0[i][:, :xt0_n],
                                xT_d[i][:, bass.ds(0, xt0_n)])
                        nc.sync.dma_start(t[:], w1_d[e][j2][i])
                        row.append(t)
                    if j2 == 0:
                        b1_sb = wpool.tile([128, 16], mybir.dt.float32,
                                           tag="b1", name="b1_sb")
                        nc.sync.dma_start(b1_sb[:], b1_d[e])
                    w1_sb.append(row)
                for j in range(16):
                    t = wpool.tile([128, D], mybir.dt.bfloat16,
                                   tag=f"w2_{j}", name=f"w2_{j}")
                    nc.sync.dma_start(t[:], w2_d[e][j])
                    w2_sb.append(t)
                return w1_sb, w2_sb, b1_sb

            def load_x(off, n):
                # gpsimd queue: decoupled from the weight/output stream on sync
                xt = []
                for i in range(4):
                    t = xpool.tile([128, TOK_BLK], mybir.dt.bfloat16,
                                   tag=f"xt_{i}", name=f"xt_{i}")
                    nc.gpsimd.dma_start(t[:, :n], xT_d[i][:, bass.ds(off, n)])
                    xt.append(t)
                return xt

            def stage1(wset, xt, n):
                w1_sb, _, b1_sb = wset
                hT = hpool.tile([128, 16 * TOK_BLK], mybir.dt.bfloat16,
                                tag="hT", name="hT")
                for j in range(16):
                    ph = phpool.tile([128, TOK_BLK], mybir.dt.float32,
                                     tag="ph", name="ph")
                    for i in range(4):
                        nc.tensor.matmul(
                            ph[:, :n],
                            w1_sb[j // 4][i][:, bass.ts(j % 4, 128)],
                            xt[i][:, :n],
                            start=(i == 0),
                            stop=(i == 3),
                        )
                    nc.scalar.activation(
                        hT[:, bass.ds(j * TOK_BLK, n)],
                        ph[:, :n],
                        mybir.ActivationFunctionType.Relu,
                        bias=b1_sb[:, j:j + 1],
                    )
                return hT

            def stage2(wset, hT, off, n, roff):
                _, w2_sb, _ = wset
                for m in range((n + 127) // 128):
                    p = min(128, n - m * 128)  # partial partitions at tail
                    py = pypool.tile([128, D], mybir.dt.float32, tag="py",
                                     name="py")
                    for j in range(16):
                        nc.tensor.matmul(
                            py[:p, :],
                            hT[:, bass.ds(j * TOK_BLK + m * 128, p)],
                            w2_sb[j][:],
                            start=(j == 0),
                            stop=(j == 15),
                        )
                    ysb = ypool.tile([128, D], mybir.dt.float32, tag="ysb",
                                     name="ysb")
                    nc.vector.tensor_copy(ysb[:p, :], py[:p, :])
                    nc.sync.dma_start(
                        y_d[bass.ds(off + m * 128, p), :], ysb[:p, :]
                    )

            # software pipeline: S1(k+1) emitted before S2(k); weights for
            # expert e+1 requested at e's last block (slot rotation makes the
            # DMA wait until slot e-1 is drained).
            xt0_n = min(TOK_BLK, shares[0])
            xt0 = [xpool.tile([128, TOK_BLK], mybir.dt.bfloat16,
                              tag=f"xt_{i}", name=f"xt0_{i}")
                   for i in range(4)]
            wsets = {0: load_expert(0, xt0=xt0)}

            prev = None  # (wset, hT, off, n, roff)
            for k, (e, off, n, roff) in enumerate(blocks):
                if e not in wsets:
                    wsets = {e: load_expert(e)} | {
                        ee: ws for ee, ws in wsets.items() if ee == e - 1
                    }
                xt = xt0 if k == 0 else load_x(off, n)
                hT = stage1(wsets[e], xt, n)
                if prev is not None:
                    stage2(*prev)
                prev = (wsets[e], hT, off, n, roff)
            stage2(*prev)

    nc.compile()
    return nc


def _route_host(t, Wr, br):
    logits = t @ Wr + br
    m = logits.max(axis=1, keepdims=True)
    eg = np.exp(logits - m)
    gates = eg / eg.sum(axis=1, keepdims=True)
    order = np.argsort(-gates, axis=1, kind="stable")[:, :TOPK]
    topv = np.take_along_axis(gates, order, axis=1)
    wts = topv / topv.sum(axis=1, keepdims=True)
    return order, wts.astype(np.float32)


def kernel(x, Wr, br, W1, b1, W2, b2):
    global LAST_RESULTS
    x = np.asarray(x, np.float32)
    Wr = np.asarray(Wr, np.float32)
    br = np.asarray(br, np.float32)
    W1 = np.asarray(W1, np.float32)
    b1 = np.asarray(b1, np.float32)
    W2 = np.asarray(W2, np.float32)
    b2 = np.asarray(b2, np.float32)

    orig_shape = x.shape
    t = x.reshape(-1, D)
    T = t.shape[0]

    order, wts = _route_host(t, Wr, br)

    idx_e, wt_e = [], []
    for e in range(E):
        rows, cols = np.nonzero(order == e)
        idx_e.append(rows)
        wt_e.append(wts[rows, cols])
    counts = [len(r) for r in idx_e]
    shares = tuple(int(-(-counts[e] // N_CORES)) for e in range(E))

    nc = _compiled_cache.get(shares)
    if nc is None:
        nc = _build_kernel(shares)
        _compiled_cache[shares] = nc
    blocks, C, R = _block_list(shares)

    w1p = np.ascontiguousarray(
        W1.reshape(E, 4, 128, 4, 512).transpose(0, 3, 1, 2, 4)
    ).astype(_BF16)
    w2p = np.ascontiguousarray(W2).reshape(E, 16, 128, D).astype(_BF16)
    b1p = np.ascontiguousarray(b1.reshape(E, 16, 128).transpose(0, 2, 1))

    in_maps = []
    core_maps = []  # per core: (idx[C], wt[C], nvalid per expert)
    for c in range(N_CORES):
        idx = np.zeros(C, np.int64)
        wpad = np.zeros(C, np.float32)
        nval = []
        off = 0
        for e in range(E):
            sel = idx_e[e][c::N_CORES]
            ne = len(sel)
            idx[off:off + ne] = sel
            wpad[off:off + ne] = wt_e[e][c::N_CORES]
            nval.append(ne)
            off += shares[e]
        xe_T = np.ascontiguousarray(t[idx].T)
        in_maps.append({
            "xT": xe_T.reshape(4, 128, C).astype(_BF16),
            "w1": w1p,
            "w2": w2p,
            "b1": b1p,
        })
        core_maps.append((idx, wpad, nval))

    LAST_RESULTS = bass_utils.run_bass_kernel_spmd(
        nc, in_maps, core_ids=list(range(N_CORES))
    )

    out = np.zeros((T, D), np.float32)
    for c in range(N_CORES):
        res = LAST_RESULTS.results[c]
        ye = np.asarray(res["y"], np.float32)
        idx, wpad, nval = core_maps[c]
        off = 0
        for e in range(E):
            ne = nval[e]
            if ne:
                rows = idx[off:off + ne]
                w = wpad[off:off + ne]
                out[rows] += w[:, None] * ye[off:off + ne] + np.outer(w, b2[e])
            off += shares[e]
    return out.reshape(orig_shape)


# revision 23
# speedup vs baseline: 1.1927x; 1.1927x over previous
"""MoE (top-2 of 8 experts) Trainium2 kernel.

Strategy: token-balanced expert loop over 8 NeuronCores. The router
(softmax + top-2 + renormalize) runs on host in f32 numpy, exactly
mirroring the jax reference semantics (stable argsort == lax.top_k
tie-breaking). Every core loops over all 8 experts; expert e's routed
tokens are dealt stride-8 across cores, so each core processes exactly
v_e = ceil(count_e/8) tokens of expert e — per-core work is balanced to
<0.1% regardless of routing skew. Expert weights are DMA-streamed per
expert (bf16, double-buffered, hidden under the ~55us of matmul per
expert). Core math per expert (combine-weight scaling and the w*b2
rank-1 term are applied on host, exactly, during the scatter-add):

    y = relu(x @ W1[e] + b1[e]) @ W2[e]

Matmuls run in bf16 on the PE array with f32 PSUM accumulation; b1-add +
relu is fused into one ScalarE activation. Tokens are processed in
blocks of <=512 (one PSUM bank); the sub-128 tail of a block runs as a
partial-partition stage-2 group. Stage 1 of block k+1 is emitted before
stage 2 of block k so the PE stream never stalls on the relu drain. w1
is fed in [128,512] chunks so the first matmul starts after ~256KB of
DMA instead of 2MB; x blocks ride the gpsimd DMA queue, decoupled from
the weight/output stream on the sync queue.

Layouts (host-prepped so the device only does natural slices):
  xT  [4,128,C]        bf16  x_gathered^T as (d//128, d%128, slot)
  w1  [E,4,4,128,512]  bf16  W1 as (e, f//512, d//128, d%128, f%512)
  w2  [E,16,128,D]     bf16  W2 as (e, f//128, f%128, d)
  b1  [E,128,16]       f32   b1 as (e, f%128, f//128) -> ACT bias column
  y   [C,D]            f32   output slots, [slot, d]
"""

import os
import sys
import numpy as np
import ml_dtypes

import concourse.bass as bass
import concourse.mybir as mybir
import concourse.tile as tile
from concourse import bacc, bass_utils

# If BASS_TRACE is set, run_bass_kernel_spmd's axon path imports
# antenv.axon_hooks, which this image's antenv lacks (boot degrades
# silently). Synthesize it from trn_agent_boot so tracing works instead
# of crashing; if that fails, disable tracing.
if os.environ.get("BASS_TRACE") and "antenv.axon_hooks" not in sys.modules:
    try:
        import types
        from trn_agent_boot.trn_boot import _ntff_profile_via_ctypes

        _hooks = types.ModuleType("antenv.axon_hooks")
        _hook = _ntff_profile_via_ctypes("/opt/axon/libaxon_pjrt.so")
        _hooks.get_axon_ntff_profile_hook = lambda: _hook
        _hooks.set_axon_ntff_profile_hook = lambda h: None
        sys.modules["antenv.axon_hooks"] = _hooks
        if not getattr(bass_utils.upload_artifacts, "_local", False):
            bass_utils.upload_artifacts = lambda tmpdir: f"local:{tmpdir}"
            bass_utils.upload_artifacts._local = True
    except Exception:
        os.environ["BASS_NEVER_TRACE"] = "1"

B, S, D, F, E, TOPK = 64, 512, 512, 2048, 8, 2
N_CORES = 8
TOK_BLK = 512

_BF16 = ml_dtypes.bfloat16
_compiled_cache: dict[tuple, "bacc.Bacc"] = {}
LAST_RESULTS = None  # test harness reads exec_time_ns / profile from here


def _block_list(shares):
    """Compile-time blocks: (expert, slot_off, n_tok), n_tok <= 512."""
    blocks = []
    off = 0
    for e, sh in enumerate(shares):
        left = sh
        while left > 0:
            n = min(TOK_BLK, left)
            blocks.append((e, off, n))
            off += n
            left -= n
    return blocks, off


def _build_kernel(shares) -> "bacc.Bacc":
    blocks, C = _block_list(shares)
    nc = bacc.Bacc("TRN2", target_bir_lowering=False, debug=False,
                   num_devices=N_CORES)

    xT_d = nc.dram_tensor("xT", [4, 128, C], mybir.dt.bfloat16,
                          kind="ExternalInput")
    w1_d = nc.dram_tensor("w1", [E, 4, 4, 128, 512], mybir.dt.bfloat16,
                          kind="ExternalInput")
    w2_d = nc.dram_tensor("w2", [E, 16, 128, D], mybir.dt.bfloat16,
                          kind="ExternalInput")
    b1_d = nc.dram_tensor("b1", [E, 128, 16], mybir.dt.float32,
                          kind="ExternalInput")
    y_d = nc.dram_tensor("y", [C, D], mybir.dt.float32,
                         kind="ExternalOutput")

    with tile.TileContext(nc) as tc:
        with (
            tc.tile_pool(name="wpool", bufs=2) as wpool,
            tc.tile_pool(name="xin", bufs=6) as xpool,
            tc.tile_pool(name="hbuf", bufs=2) as hpool,
            tc.tile_pool(name="yout", bufs=3) as ypool,
            tc.tile_pool(name="ph", bufs=3, space="PSUM") as phpool,
            tc.tile_pool(name="py", bufs=3, space="PSUM") as pypool,
        ):
            def load_expert(e, xt0=None):
                w1_sb, w2_sb = [], []
                for j2 in range(4):
                    row = []
                    for i in range(4):
                        t = wpool.tile([128, 512], mybir.dt.bfloat16,
                                       tag=f"w1_{j2}_{i}", name=f"w1_{j2}_{i}")
                        # first expert: first x block goes down the gpsimd
                        # queue in parallel with w1 on sync, so the PE's
                        # first matmul (needs only w1_00 + xt_0) starts early
                        if xt0 is not None and j2 == 0:
                            nc.gpsimd.dma_start(
                                xt0[i][:, :xt0_n],
                                xT_d[i][:, bass.ds(0, xt0_n)])
                        nc.sync.dma_start(t[:], w1_d[e][j2][i])
                        row.append(t)
                    if j2 == 0:
                        b1_sb = wpool.tile([128, 16], mybir.dt.float32,
                                           tag="b1", name="b1_sb")
                        nc.sync.dma_start(b1_sb[:], b1_d[e])
                    w1_sb.append(row)
                for j in range(16):
                    t = wpool.tile([128, D], mybir.dt.bfloat16,
                                   tag=f"w2_{j}", name=f"w2_{j}")
                    nc.sync.dma_start(t[:], w2_d[e][j])
                    w2_sb.append(t)
                return w1_sb, w2_sb, b1_sb

            def load_x(off, n):
                # gpsimd queue: decoupled from the weight/output stream on sync
                xt = []
                for i in range(4):
                    t = xpool.tile([128, TOK_BLK], mybir.dt.bfloat16,
                                   tag=f"xt_{i}", name=f"xt_{i}")
                    nc.gpsimd.dma_start(t[:, :n], xT_d[i][:, bass.ds(off, n)])
                    xt.append(t)
                return xt

            def stage1(wset, xt, n):
                w1_sb, _, b1_sb = wset
                hT = hpool.tile([128, 16 * TOK_BLK], mybir.dt.bfloat16,
                                tag="hT", name="hT")
                for j in range(16):
                    ph = phpool.tile([128, TOK_BLK], mybir.dt.float32,
                                     tag="ph", name="ph")
                    for i in range(4):
                        nc.tensor.matmul(
                            ph[:, :n],
                            w1_sb[j // 4][i][:, bass.ts(j % 4, 128)],
                            xt[i][:, :n],
                            start=(i == 0),
                            stop=(i == 3),
                        )
                    nc.scalar.activation(
                        hT[:, bass.ds(j * TOK_BLK, n)],
                        ph[:, :n],
                        mybir.ActivationFunctionType.Relu,
                        bias=b1_sb[:, j:j + 1],
                    )
                return hT

            def stage2(wset, hT, off, n):
                _, w2_sb, _ = wset
                for m in range((n + 127) // 128):
                    p = min(128, n - m * 128)  # partial partitions at tail
                    py = pypool.tile([128, D], mybir.dt.float32, tag="py",
                                     name="py")
                    for j in range(16):
                        nc.tensor.matmul(
                            py[:p, :],
                            hT[:, bass.ds(j * TOK_BLK + m * 128, p)],
                            w2_sb[j][:],
                            start=(j == 0),
                            stop=(j == 15),
                        )
                    ysb = ypool.tile([128, D], mybir.dt.float32, tag="ysb",
                                     name="ysb")
                    nc.vector.tensor_copy(ysb[:p, :], py[:p, :])
                    nc.sync.dma_start(
                        y_d[bass.ds(off + m * 128, p), :], ysb[:p, :]
                    )

            # software pipeline: S1(k+1) emitted before S2(k); weights for
            # expert e+1 requested at e's last block (slot rotation makes the
            # DMA wait until slot e-1 is drained).
            xt0_n = min(TOK_BLK, shares[0])
            xt0 = [xpool.tile([128, TOK_BLK], mybir.dt.bfloat16,
                              tag=f"xt_{i}", name=f"xt0_{i}")
                   for i in range(4)]
            wsets = {0: load_expert(0, xt0=xt0)}

            prev = None  # (wset, hT, off, n)
            for k, (e, off, n) in enumerate(blocks):
                if e not in wsets:
                    wsets = {e: load_expert(e)} | {
                        ee: ws for ee, ws in wsets.items() if ee == e - 1
                    }
                xt = xt0 if k == 0 else load_x(off, n)
                hT = stage1(wsets[e], xt, n)
                if prev is not None:
                    stage2(*prev)
                prev = (wsets[e], hT, off, n)
            stage2(*prev)

    nc.compile()
    return nc


def _route_host(t, Wr, br):
    logits = t @ Wr + br
    m = logits.max(axis=1, keepdims=True)
    eg = np.exp(logits - m)
    gates = eg / eg.sum(axis=1, keepdims=True)
    order = np.argsort(-gates, axis=1, kind="stable")[:, :TOPK]
    topv = np.take_along_axis(gates, order, axis=1)
    wts = topv / topv.sum(axis=1, keepdims=True)
    return order, wts.astype(np.float32)


def kernel(x, Wr, br, W1, b1, W2, b2):
    global LAST_RESULTS
    x = np.asarray(x, np.float32)
    Wr = np.asarray(Wr, np.float32)
    br = np.asarray(br, np.float32)
    W1 = np.asarray(W1, np.float32)
    b1 = np.asarray(b1, np.float32)
    W2 = np.asarray(W2, np.float32)
    b2 = np.asarray(b2, np.float32)

    orig_shape = x.shape
    t = x.reshape(-1, D)
    T = t.shape[0]

    order, wts = _route_host(t, Wr, br)

    idx_e, wt_e = [], []
    for e in range(E):
        rows, cols = np.nonzero(order == e)
        idx_e.append(rows)
        wt_e.append(wts[rows, cols])
    counts = [len(r) for r in idx_e]
    shares = tuple(int(-(-counts[e] // N_CORES)) for e in range(E))

    nc = _compiled_cache.get(shares)
    if nc is None:
        nc = _build_kernel(shares)
        _compiled_cache[shares] = nc
    C = int(sum(shares))

    w1p = np.ascontiguousarray(
        W1.reshape(E, 4, 128, 4, 512).transpose(0, 3, 1, 2, 4)
    ).astype(_BF16)
    w2p = np.ascontiguousarray(W2).reshape(E, 16, 128, D).astype(_BF16)
    b1p = np.ascontiguousarray(b1.reshape(E, 16, 128).transpose(0, 2, 1))

    in_maps = []
    core_maps = []  # per core: (idx[C], wt[C], nvalid per expert)
    for c in range(N_CORES):
        idx = np.zeros(C, np.int64)
        wpad = np.zeros(C, np.float32)
        nval = []
        off = 0
        for e in range(E):
            sel = idx_e[e][c::N_CORES]
            ne = len(sel)
            idx[off:off + ne] = sel
            wpad[off:off + ne] = wt_e[e][c::N_CORES]
            nval.append(ne)
            off += shares[e]
        xe_T = np.ascontiguousarray(t[idx].T)
        in_maps.append({
            "xT": xe_T.reshape(4, 128, C).astype(_BF16),
            "w1": w1p,
            "w2": w2p,
            "b1": b1p,
        })
        core_maps.append((idx, wpad, nval))

    LAST_RESULTS = bass_utils.run_bass_kernel_spmd(
        nc, in_maps, core_ids=list(range(N_CORES))
    )

    out = np.zeros((T, D), np.float32)
    for c in range(N_CORES):
        res = LAST_RESULTS.results[c]
        ye = np.asarray(res["y"], np.float32)
        idx, wpad, nval = core_maps[c]
        off = 0
        for e in range(E):
            ne = nval[e]
            if ne:
                rows = idx[off:off + ne]
                w = wpad[off:off + ne]
                out[rows] += w[:, None] * ye[off:off + ne] + np.outer(w, b2[e])
            off += shares[e]
    return out.reshape(orig_shape)
